# revision 1
# baseline (speedup 1.0000x reference)
"""Trainium2 Bass kernel for windowed (block-diagonal) multi-head video attention.

Problem: x:[2,8192,1024] -> qkv proj -> 3D-window (2,8,8) attention over a
(8,32,32) token grid, 16 heads x 64 dim -> out proj -> [2,8192,1024].

Sharding: 8 cores, data-parallel over (batch, t-window-group).  Token order is
(t,h,w)-major, so the slab x[b, it*2048:(it+1)*2048, :] is contiguous and holds
exactly the 16 independent (h,w)-windows with t in {2it, 2it+1}.  Each core:
  - DMA-gathers each window's 128 tokens as a [128,1024] tile (strided AP)
  - PE-transposes x_win -> x^T (contraction dim on partitions)
  - QKV projection: Q,K produced head-transposed [oc,tok]; V token-major with a
    per-head ones column appended (65-stride layout)
  - S^T = K_h Q_h^T per head (K=64), exp on ACT, A·V matmul where the ones row
    yields the softmax denominator for free; normalize with reciprocal +
    gpsimd partition-broadcast + DVE multiply
  - out projection, DMA-scatter back to token order
Weights are pre-transposed on the host; biases (zero in this problem) are
supported via rank-1 (K=1) accumulation matmuls, compiled only when nonzero.
"""

import sys

for _p in ("/opt/trn_rl_repo",):
    if _p not in sys.path:
        sys.path.insert(0, _p)

import numpy as np

B, T, H, W = 2, 8, 32, 32
C, NH, HD = 1024, 16, 64
WT, WH, WW = 2, 8, 8
N = T * H * W              # 8192 tokens
SCALE = HD ** -0.5
NCORES = 8
SLAB = N // (T // WT)      # 2048 tokens per (b, it) slab
NWIN = (H // WH) * (W // WW)   # 16 windows per slab
M = WT * WH * WW           # 128 tokens per window
KC = C // 128              # 8 contraction chunks

_BUILD_CACHE = {}


def _split_drain_waits(nc, mybir, cap=1, event_cap=2):
    """This walrus build accepts only one sem wait per TPB instruction
    (Tile's scheduler attaches up to 3).  Move the excess onto
    InstEventSemaphore carriers (which hold 2) inserted right before the
    over-subscribed instruction on the same engine — the engine blocks on the
    carriers first, so semantics are unchanged."""
    for f in nc.m.functions:
        for bb in f.blocks:
            i = 0
            while i < len(bb.instructions):
                ins = bb.instructions[i]
                si = ins.sync_info
                my_cap = (
                    event_cap
                    if type(ins).__name__ == "InstEventSemaphore"
                    else cap
                )
                if si is not None and si.on_wait and len(si.on_wait) > my_cap:
                    waits = list(si.on_wait)
                    si.on_wait = waits[:my_cap]
                    extra = waits[my_cap:]
                    carriers = []
                    while extra:
                        chunk, extra = extra[:event_cap], extra[event_cap:]
                        ev = mybir.InstEventSemaphore(
                            name=f"I-{nc.next_id()}-waitsplit", ins=[], outs=[]
                        )
                        ev.engine = ins.engine
                        ev.sync_info = mybir.SyncInfo(
                            on_wait=list(chunk), on_update=[]
                        )
                        nc.register_instruction(ev)
                        carriers.append(ev)
                    bb.instructions[i:i] = carriers
                    i += len(carriers)
                i += 1


def _build(has_qkvb, has_projb, use_f32r=True):
    import concourse.bass as bass
    import concourse.tile as tile
    from concourse import mybir
    f32 = mybir.dt.float32
    fpr = mybir.dt.float32r if use_f32r else f32

    nc = bass.Bass("TRN2", target_bir_lowering=False, debug=False)
    xs = nc.dram_tensor("xs", [SLAB, C], fpr, kind="ExternalInput")
    # weight dtype f32r: same 4-byte layout, PE rounds on read (tf32-like)
    wqkvT = nc.dram_tensor("wqkvT", [C, 3 * C], fpr, kind="ExternalInput")
    projT = nc.dram_tensor("projT", [C, C], fpr, kind="ExternalInput")
    if has_qkvb:
        qkvb = nc.dram_tensor("qkvb", [1, 3 * C], fpr, kind="ExternalInput")
    if has_projb:
        projb = nc.dram_tensor("projb", [1, C], fpr, kind="ExternalInput")
    ident_d = nc.dram_tensor("ident", [128, 128], fpr, kind="ExternalInput")
    out = nc.dram_tensor("out", [SLAB, C], f32, kind="ExternalOutput")

    # window gather/scatter views: slab token idx = tt*1024 + hh*32 + ww in a
    # [2, (4,8), (4,8)] = (tt, ih hh, iw ww) decomposition; window = (ih, iw)
    xs_v = xs.ap().rearrange(
        "(tt ih hh iw ww) c -> ih iw tt hh ww c", tt=WT, ih=4, hh=WH, iw=4, ww=WW
    )
    out_v = out.ap().rearrange(
        "(tt ih hh iw ww) c -> ih iw tt hh ww c", tt=WT, ih=4, hh=WH, iw=4, ww=WW
    )

    # windows processed in pairs: tok dim = 256 so the f32r matmuls hit the
    # 1 cyc/row regime (ap_size >= 256); attention blocks stay per-window
    GW = 2
    TOKG = 128 * GW

    with tile.TileContext(nc) as tc:
        with (
            tc.tile_pool(name="wq", bufs=1) as wq_pool,
            tc.tile_pool(name="wp", bufs=1) as wp_pool,
            tc.tile_pool(name="const", bufs=1) as const_pool,
            tc.tile_pool(name="xw", bufs=2 if not (has_qkvb or has_projb) else 1) as xw_pool,
            tc.tile_pool(name="xT", bufs=1) as xT_pool,
            tc.tile_pool(name="qk", bufs=1) as qk_pool,
            tc.tile_pool(name="v65", bufs=1) as v_pool,
            tc.tile_pool(name="E", bufs=3 if not (has_qkvb or has_projb) else 2) as e_pool,
            tc.tile_pool(name="rR", bufs=2 if not (has_qkvb or has_projb) else 1) as r_pool,
            tc.tile_pool(name="owT", bufs=1) as ow_pool,
            tc.tile_pool(name="o", bufs=1) as o_pool,
            tc.tile_pool(name="psA", bufs=4, space="PSUM") as psA,
            tc.tile_pool(name="psB", bufs=4, space="PSUM") as psB,
        ):
            # identity via DMA: make_identity runs on GpSimd, whose cold
            # start would gate the first PE transpose
            ident = const_pool.tile([128, 128], fpr)
            nc.scalar.dma_start(ident[:], ident_d.ap())
            ones_col = const_pool.tile([128, GW * NH], f32)
            nc.vector.memset(ones_col[:], 1.0)
            ones64f = const_pool.tile([1, 64], f32)
            nc.vector.memset(ones64f[:], 1.0)
            ones64 = const_pool.tile([1, 64], fpr)
            nc.scalar.copy(ones64[:], ones64f[:])

            wq_sb = wq_pool.tile([128, KC, 3 * C], fpr)
            wq_src = wqkvT.ap().rearrange("(k p) o -> p k o", p=128)
            for k in range(KC):
                nc.sync.dma_start(
                    wq_sb[:, k : k + 1, 0 : 2 * C], wq_src[:, k : k + 1, 0 : 2 * C]
                )
            for k in range(KC):
                nc.sync.dma_start(
                    wq_sb[:, k : k + 1, 2 * C :], wq_src[:, k : k + 1, 2 * C :]
                )
            wp_sb = wp_pool.tile([128, KC, C], fpr)
            wp_src = projT.ap().rearrange("(k p) o -> p k o", p=128)
            for k in range(KC):
                nc.sync.dma_start(wp_sb[:, k : k + 1, :], wp_src[:, k : k + 1, :])
            if has_qkvb or has_projb:
                onesf = const_pool.tile([1, TOKG], f32)
                nc.vector.memset(onesf[:], 1.0)
                ones = const_pool.tile([1, TOKG], fpr)
                nc.scalar.copy(ones[:], onesf[:])
            if has_qkvb:
                qkvb_sb = const_pool.tile([1, 3 * C], fpr)
                nc.sync.dma_start(qkvb_sb[:], qkvb.ap())
            if has_projb:
                projb_sb = const_pool.tile([1, C], fpr)
                nc.sync.dma_start(projb_sb[:], projb.ap())

            for grp in range(NWIN // GW):
                wins = [(divmod(GW * grp + w, 4)) for w in range(GW)]

                # 1+2) per window: gather tokens, PE-transpose into the
                # group x^T tile [c-chunk partitions, (chunk, tok)] (f32r)
                xT = xT_pool.tile([128, KC, TOKG], fpr)
                for w, (ih, iw) in enumerate(wins):
                    xw = xw_pool.tile([128, C], fpr)
                    for tt in range(WT):
                        nc.scalar.dma_start(
                            xw[64 * tt : 64 * (tt + 1), :], xs_v[ih, iw, tt]
                        )
                    for tb in range(2):
                        ps = psA.tile([128, 512], fpr, tag="psA")
                        for j in range(4):
                            jj = 4 * tb + j
                            nc.tensor.transpose(
                                ps[:, 128 * j : 128 * (j + 1)],
                                xw[:, 128 * jj : 128 * (jj + 1)],
                                ident[:],
                            )
                        psv = ps[:].rearrange("p (c t) -> p c t", t=128)
                        with nc.allow_low_precision(reason="f32r eviction"):
                            nc.vector.tensor_copy(
                                xT[:].rearrange("p k (g t) -> p k g t", g=GW)[
                                    :, 4 * tb : 4 * tb + 4, w, :
                                ],
                                psv[:],
                            )

                # 3) Q,K head-transposed: psum bank [oc 128, tok 256] x2 chunks.
                # Evict to 64-partition per-head layout (slot 2c+parity) so S
                # matmuls never use partition-base-64 operands (mixing base-0
                # and base-64 matmul operands hangs trn2).  qkT is f32r so the
                # S matmuls run as a single (rounded) pass instead of fp32's
                # HI+LO pair.
                qkT = qk_pool.tile([64, 4 * KC, TOKG], fpr)
                qkTv = qkT[:].rearrange("p (s two) t -> p s two t", two=2)
                for bank in (0, 4, 1, 5, 2, 6, 3, 7):
                    ps = psA.tile([128, 512], f32, tag="psA")
                    for sub in range(2):
                        oc = 2 * bank + sub
                        for k in range(KC):
                            nc.tensor.matmul(
                                ps[:, TOKG * sub : TOKG * (sub + 1)],
                                wq_sb[:, k, 128 * oc : 128 * (oc + 1)],
                                xT[:, k, :],
                                start=(k == 0),
                                stop=(k == KC - 1 and not has_qkvb),
                            )
                        if has_qkvb:
                            nc.tensor.matmul(
                                ps[:, TOKG * sub : TOKG * (sub + 1)],
                                qkvb_sb[0:1, 128 * oc : 128 * (oc + 1)],
                                ones[0:1, 0:TOKG],
                                start=False,
                                stop=True,
                            )
                    sc = SCALE if bank < 4 else 1.0
                    psv = ps[:].rearrange("p (c t) -> p c t", t=TOKG)
                    with nc.allow_low_precision(reason="f32r eviction"):
                        nc.vector.tensor_scalar_mul(
                            qkTv[:, 2 * bank : 2 * bank + 2, 0, :],
                            psv[0:64, :, :],
                            sc,
                        )
                        nc.vector.tensor_scalar_mul(
                            qkTv[:, 2 * bank : 2 * bank + 2, 1, :],
                            psv[64:128, :, :],
                            sc,
                        )

                # 4) V token-major per window, ones column per head (stride 65)
                v65 = v_pool.tile([128, GW, NH, HD + 1], fpr)
                nc.scalar.copy(
                    v65[:, :, :, HD : HD + 1],
                    ones_col[:].rearrange("p (g h) -> p g h", g=GW)[:, :, :, None],
                )
                for w in range(GW):
                    for nk in range(2):
                        ps = psA.tile([128, 512], f32, tag="psA")
                        for half in range(2):
                            lo = 2 * C + 512 * nk + 256 * half
                            for k in range(KC):
                                nc.tensor.matmul(
                                    ps[:, 256 * half : 256 * (half + 1)],
                                    xT[:].rearrange(
                                        "p k (g t) -> p k g t", g=GW
                                    )[:, k, w, :],
                                    wq_sb[:, k, lo : lo + 256],
                                    start=(k == 0),
                                    stop=(k == KC - 1 and not has_qkvb),
                                )
                            if has_qkvb:
                                nc.tensor.matmul(
                                    ps[:, 256 * half : 256 * (half + 1)],
                                    ones[0:1, 0:128],
                                    qkvb_sb[0:1, lo : lo + 256],
                                    start=False,
                                    stop=True,
                                )
                        # one strided eviction for all 8 heads of this bank
                        nc.scalar.copy(
                            v65[:, w, 8 * nk : 8 * nk + 8, 0:HD],
                            ps[:].rearrange("p (h e) -> p h e", e=HD),
                        )

                # 5+6) attention per (4-head bank, window), then out projection
                for w, (ih, iw) in enumerate(wins):
                    owT = ow_pool.tile([128, KC, 128], fpr)
                    # all 4 S banks first so exp/AV overlap the S matmuls
                    psS_banks = []
                    for hb in range(4):
                        psS = psB.tile([128, 512], f32, tag="psB")
                        for m in range(4):
                            h = 4 * hb + m
                            # S^T[kt,qt] = (K_h^T).T @ Q_h^T, K=64, base 0
                            nc.tensor.matmul(
                                psS[:, 128 * m : 128 * (m + 1)],
                                qkT[:, NH + h, 128 * w : 128 * (w + 1)],
                                qkT[:, h, 128 * w : 128 * (w + 1)],
                                start=True,
                                stop=True,
                            )
                        psS_banks.append(psS)
                    for hb in range(4):
                        E = e_pool.tile([128, 512], fpr, tag="E")
                        with nc.allow_low_precision(reason="f32r attn weights"):
                            nc.scalar.activation(
                                E[:],
                                psS_banks[hb][:],
                                mybir.ActivationFunctionType.Exp,
                            )
                        psV = psA.tile([128, 512], f32, tag="psA")
                        for m in range(4):
                            h = 4 * hb + m
                            # rows 0..63 = V^T E (unnormalized), row 64 = denom
                            nc.tensor.matmul(
                                psV[0:65, 128 * m : 128 * (m + 1)],
                                v65[:, w, h, :],
                                E[:, 128 * m : 128 * (m + 1)],
                                start=True,
                                stop=True,
                            )
                        # softmax 1/denom as exp(-ln(den)) on the ACT
                        # tables (InstReciprocal costs ~9 cyc/elem/lane and
                        # the denom row is a single-partition [1,512]);
                        # then partition-broadcast via a K=1 matmul
                        L = r_pool.tile([1, 512], f32, tag="r")
                        nc.scalar.activation(
                            L[:], psV[64:65, :], mybir.ActivationFunctionType.Ln
                        )
                        r = r_pool.tile([1, 512], fpr, tag="r")
                        with nc.allow_low_precision(reason="f32r recip"):
                            nc.scalar.activation(
                                r[:],
                                L[:],
                                mybir.ActivationFunctionType.Exp,
                                scale=-1.0,
                            )
                        Rp = psA.tile([64, 512], f32, tag="psA")
                        for half in range(2):
                            nc.tensor.matmul(
                                Rp[:, 256 * half : 256 * (half + 1)],
                                ones64[:],
                                r[0:1, 256 * half : 256 * (half + 1)],
                                start=True,
                                stop=True,
                            )
                        R = r_pool.tile([64, 512], f32, tag="R")
                        nc.scalar.copy(R[:], Rp[:])
                        for m in range(4):
                            h = 4 * hb + m
                            po = (h % 2) * 64
                            nc.vector.tensor_tensor(
                                owT[po : po + 64, h // 2, :],
                                psV[0:64, 128 * m : 128 * (m + 1)],
                                R[:, 128 * m : 128 * (m + 1)],
                                op=mybir.AluOpType.mult,
                            )

                    otile = o_pool.tile([128, C], f32)
                    for nk in range(2):
                        ps = psA.tile([128, 512], f32, tag="psA")
                        for half in range(2):
                            lo = 512 * nk + 256 * half
                            for k in range(KC):
                                nc.tensor.matmul(
                                    ps[:, 256 * half : 256 * (half + 1)],
                                    owT[:, k, :],
                                    wp_sb[:, k, lo : lo + 256],
                                    start=(k == 0),
                                    stop=(k == KC - 1 and not has_projb),
                                )
                            if has_projb:
                                nc.tensor.matmul(
                                    ps[:, 256 * half : 256 * (half + 1)],
                                    ones[0:1, 0:128],
                                    projb_sb[0:1, lo : lo + 256],
                                    start=False,
                                    stop=True,
                                )
                        nc.vector.tensor_copy(
                            otile[:, 512 * nk : 512 * (nk + 1)], ps[:]
                        )
                    for tt in range(WT):
                        nc.sync.dma_start(
                            out_v[ih, iw, tt], otile[64 * tt : 64 * (tt + 1), :]
                        )

    _split_drain_waits(nc, mybir)
    return nc


def _get_nc(has_qkvb, has_projb):
    key = (has_qkvb, has_projb)
    if key not in _BUILD_CACHE:
        _BUILD_CACHE[key] = _build(has_qkvb, has_projb)
    return _BUILD_CACHE[key]


def kernel(x, qkv_w, qkv_b, proj_w, proj_b, t, h, w, **_unused):
    from concourse.bass_utils import run_bass_kernel_spmd

    x = np.asarray(x, dtype=np.float32)
    qkv_w = np.asarray(qkv_w, dtype=np.float32)
    qkv_b = np.asarray(qkv_b, dtype=np.float32)
    proj_w = np.asarray(proj_w, dtype=np.float32)
    proj_b = np.asarray(proj_b, dtype=np.float32)
    assert x.shape == (B, N, C), x.shape
    assert int(t) == T and int(h) == H and int(w) == W

    has_qkvb = bool(np.any(qkv_b))
    has_projb = bool(np.any(proj_b))
    nc = _get_nc(has_qkvb, has_projb)

    wqkvT = np.ascontiguousarray(qkv_w.T)
    projT = np.ascontiguousarray(proj_w.T)

    in_maps = []
    for core in range(NCORES):
        b, it = divmod(core, T // WT)
        im = {
            "xs": np.ascontiguousarray(x[b, it * SLAB : (it + 1) * SLAB, :]),
            "wqkvT": wqkvT,
            "projT": projT,
            "ident": np.eye(128, dtype=np.float32),
        }
        if has_qkvb:
            im["qkvb"] = qkv_b.reshape(1, 3 * C)
        if has_projb:
            im["projb"] = proj_b.reshape(1, C)
        in_maps.append(im)

    res = run_bass_kernel_spmd(nc, in_maps, core_ids=list(range(NCORES)))

    y = np.empty((B, N, C), dtype=np.float32)
    for core in range(NCORES):
        b, it = divmod(core, T // WT)
        y[b, it * SLAB : (it + 1) * SLAB, :] = res.results[core]["out"]
    return y



# revision 3
# speedup vs baseline: 1.4257x; 1.4257x over previous
"""Trainium2 Bass kernel for windowed (block-diagonal) multi-head video attention.

Problem: x:[2,8192,1024] -> qkv proj -> 3D-window (2,8,8) attention over a
(8,32,32) token grid, 16 heads x 64 dim -> out proj -> [2,8192,1024].

Sharding: 8 cores, data-parallel over (batch, t-window-group); the slab
x[b, it*2048:(it+1)*2048, :] holds the 16 independent (h,w)-windows with
t in {2it, 2it+1}.

This version runs the whole datapath in bf16 on the PE (1 cyc/col at any
ap size, vs f32r's 4 cyc/col below ap=256, so the ap=128 attention matmuls
are 4x faster) and moves all layout work to the host:
  - x is pre-transposed and window-gathered on the host into
    [group, 128 c-part, KC, 512 tok] so the kernel needs no PE transposes
    and no strided gather DMAs; qkv/proj weights are pre-chunked so each
    oc-chunk is one contiguous DMA that arrives in consumption order.
  - windows processed in groups of GW=4 (tok dim 512 = full PSUM bank).
  - attention: S^T = K_h Q_h^T per head (bf16), exp on ACT, A.V matmul with
    a per-head ones column producing the softmax denominator for free; the
    16 head-blocks of a window share one 4-bank PSUM tile so the denominator
    row is a single [1,2048] AP -> one Ln + one exp(-x) per window;
    1/den is partition-broadcast via a K=1 matmul and applied on DVE.
  - window finish (recip-broadcast, normalize, out-proj) is software-
    pipelined one window behind S/AV so the PE never waits on the ACT chain.
Output is written window-major and un-permuted on the host.
"""

import sys

for _p in ("/opt/trn_rl_repo",):
    if _p not in sys.path:
        sys.path.insert(0, _p)

import numpy as np
import ml_dtypes

B, T, H, W = 2, 8, 32, 32
C, NH, HD = 1024, 16, 64
WT, WH, WW = 2, 8, 8
N = T * H * W              # 8192 tokens
SCALE = HD ** -0.5
NCORES = 8
SLAB = N // (T // WT)      # 2048 tokens per (b, it) slab
NWIN = (H // WH) * (W // WW)   # 16 windows per slab
M = WT * WH * WW           # 128 tokens per window
KC = C // 128              # 8 contraction chunks
GW = 4                     # windows per group
NGRP = NWIN // GW
TOKG = M * GW              # 512
NOC = 3 * C // 128         # 24 qkv output chunks

_BUILD_CACHE = {}
bf16 = ml_dtypes.bfloat16


def _split_drain_waits(nc, mybir, cap=1, event_cap=2):
    """This walrus build accepts only one sem wait per TPB instruction
    (Tile's scheduler attaches up to 3).  Move the excess onto
    InstEventSemaphore carriers (which hold 2) inserted right before the
    over-subscribed instruction on the same engine — the engine blocks on the
    carriers first, so semantics are unchanged."""
    for f in nc.m.functions:
        for bb in f.blocks:
            i = 0
            while i < len(bb.instructions):
                ins = bb.instructions[i]
                si = ins.sync_info
                my_cap = (
                    event_cap
                    if type(ins).__name__ == "InstEventSemaphore"
                    else cap
                )
                if si is not None and si.on_wait and len(si.on_wait) > my_cap:
                    waits = list(si.on_wait)
                    si.on_wait = waits[:my_cap]
                    extra = waits[my_cap:]
                    carriers = []
                    while extra:
                        chunk, extra = extra[:event_cap], extra[event_cap:]
                        ev = mybir.InstEventSemaphore(
                            name=f"I-{nc.next_id()}-waitsplit", ins=[], outs=[]
                        )
                        ev.engine = ins.engine
                        ev.sync_info = mybir.SyncInfo(
                            on_wait=list(chunk), on_update=[]
                        )
                        nc.register_instruction(ev)
                        carriers.append(ev)
                    bb.instructions[i:i] = carriers
                    i += len(carriers)
                i += 1


def _build(has_qkvb, has_projb):
    import concourse.bass as bass
    import concourse.tile as tile
    from concourse import mybir
    f32 = mybir.dt.float32
    fpr = mybir.dt.float32r
    bf = mybir.dt.bfloat16

    nc = bass.Bass("TRN2", target_bir_lowering=False, debug=False)
    # host-packed inputs (see _pack_* below)
    xs = nc.dram_tensor("xs", [NGRP, 128, KC, TOKG], bf, kind="ExternalInput")
    wqkv = nc.dram_tensor("wqkv", [NOC, 128, KC, 128], bf, kind="ExternalInput")
    wproj = nc.dram_tensor("wproj", [128, KC, C], bf, kind="ExternalInput")
    if has_qkvb:
        qkvb = nc.dram_tensor("qkvb", [1, 3 * C], bf, kind="ExternalInput")
    if has_projb:
        projb = nc.dram_tensor("projb", [1, C], bf, kind="ExternalInput")
    outd = nc.dram_tensor("out", [NWIN, M, C], f32, kind="ExternalOutput")

    Exp = mybir.ActivationFunctionType.Exp
    Ln = mybir.ActivationFunctionType.Ln

    with tile.TileContext(nc) as tc:
        with (
            tc.tile_pool(name="wq", bufs=1) as wq_pool,
            tc.tile_pool(name="wp", bufs=1) as wp_pool,
            tc.tile_pool(name="xTp", bufs=1) as xT_pool,
            tc.tile_pool(name="const", bufs=1) as const_pool,
            tc.tile_pool(name="qk", bufs=1) as qk_pool,
            tc.tile_pool(name="v65", bufs=2) as v_pool,
            tc.tile_pool(name="E", bufs=2) as e_pool,
            tc.tile_pool(name="r", bufs=1) as r_pool,
            tc.tile_pool(name="owT", bufs=2) as ow_pool,
            tc.tile_pool(name="o", bufs=2) as o_pool,
            tc.tile_pool(name="psA", bufs=2, space="PSUM") as psA,
            tc.tile_pool(name="psB", bufs=2, space="PSUM") as psB,
            tc.tile_pool(name="psV", bufs=1, space="PSUM") as psV_pool,
        ):
            wq_sb = wq_pool.tile([128, KC, 3 * C], bf)
            # oc-chunks in consumption order: QK chunks 0..15 first, V 16..23
            for oc in range(NOC):
                nc.sync.dma_start(
                    wq_sb[:, :, 128 * oc : 128 * (oc + 1)], wqkv.ap()[oc]
                )
            xT = xT_pool.tile([128, NGRP, KC, TOKG], bf)
            for g in range(NGRP):
                nc.scalar.dma_start(xT[:, g], xs.ap()[g])
            wp_sb = wp_pool.tile([128, KC, C], bf)
            nc.sync.dma_start(wp_sb[:], wproj.ap())

            onesf = const_pool.tile([1, TOKG], f32)
            nc.vector.memset(onesf[:], 1.0)
            ones64 = const_pool.tile([1, 64], fpr)
            with nc.allow_low_precision(reason="ones"):
                nc.scalar.copy(ones64[:], onesf[0:1, 0:64])
            ones_col = const_pool.tile([128, GW * NH], f32)
            nc.vector.memset(ones_col[:], 1.0)
            if has_qkvb or has_projb:
                ones_tok = const_pool.tile([1, TOKG], bf)
                with nc.allow_low_precision(reason="ones"):
                    nc.scalar.copy(ones_tok[:], onesf[:])
            if has_qkvb:
                qkvb_sb = const_pool.tile([1, 3 * C], bf)
                nc.sync.dma_start(qkvb_sb[:], qkvb.ap())
            if has_projb:
                projb_sb = const_pool.tile([1, C], bf)
                nc.sync.dma_start(projb_sb[:], projb.ap())

            def finish(win, psV, r4):
                """Normalize window win's AV block and run its out-proj."""
                owT = ow_pool.tile([128, KC, M], bf)
                psVv = psV[0:64, :].rearrange(
                    "p (hb m2 par t) -> p hb m2 par t", hb=4, m2=2, par=2
                )
                for hb in range(4):
                    Rp = psB.tile([128, 512], f32, tag="psB")
                    nc.tensor.matmul(
                        Rp[0:64, :],
                        ones64[:],
                        r4[0:1, 512 * hb : 512 * (hb + 1)],
                        start=True,
                        stop=True,
                    )
                    R = r_pool.tile([64, 512], f32, tag="R", bufs=2)
                    nc.vector.tensor_copy(R[:], Rp[0:64, :])
                    Rv = R[:].rearrange("p (m2 par t) -> p m2 par t", m2=2, par=2)
                    for par in range(2):
                        with nc.allow_low_precision(reason="bf16 owT"):
                            nc.vector.tensor_tensor(
                                owT[64 * par : 64 * (par + 1), 2 * hb : 2 * hb + 2, :],
                                psVv[:, hb, :, par, :],
                                Rv[:, :, par, :],
                                op=mybir.AluOpType.mult,
                            )
                otile = o_pool.tile([128, C], f32)
                for nk in range(2):
                    ps = psA.tile([128, 512], f32, tag="psA")
                    for k in range(KC):
                        nc.tensor.matmul(
                            ps[:],
                            owT[:, k, :],
                            wp_sb[:, k, 512 * nk : 512 * (nk + 1)],
                            start=(k == 0),
                            stop=(k == KC - 1 and not has_projb),
                        )
                    if has_projb:
                        nc.tensor.matmul(
                            ps[:],
                            ones_tok[0:1, 0:M],
                            projb_sb[0:1, 512 * nk : 512 * (nk + 1)],
                            start=False,
                            stop=True,
                        )
                    nc.vector.tensor_copy(otile[:, 512 * nk : 512 * (nk + 1)], ps[:])
                nc.sync.dma_start(outd.ap()[win], otile[:])

            prev = None
            for g in range(NGRP):
                # QKV projection, Q/K head-transposed.  qkT slot h = Q_h
                # scaled by SCALE, slot 16+h = K_h; head h lives in rows
                # (h%2)*64..+64 of oc-chunk h//2's PSUM block.
                qkT = qk_pool.tile([64, 2 * NH, TOKG], bf)
                for oc in range(16):
                    ps = psA.tile([128, 512], f32, tag="psA")
                    for k in range(KC):
                        nc.tensor.matmul(
                            ps[:],
                            wq_sb[:, k, 128 * oc : 128 * (oc + 1)],
                            xT[:, g, k, :],
                            start=(k == 0),
                            stop=(k == KC - 1 and not has_qkvb),
                        )
                    if has_qkvb:
                        nc.tensor.matmul(
                            ps[:],
                            qkvb_sb[0:1, 128 * oc : 128 * (oc + 1)],
                            ones_tok[:],
                            start=False,
                            stop=True,
                        )
                    sc = SCALE if oc < 8 else 1.0
                    base = 2 * oc  # Q slots 0..15, K slots 16..31
                    with nc.allow_low_precision(reason="bf16 qkT"):
                        nc.vector.tensor_scalar_mul(
                            qkT[:, base, :], ps[0:64, :], sc
                        )
                        nc.vector.tensor_scalar_mul(
                            qkT[:, base + 1, :], ps[64:128, :], sc
                        )

                v65 = v_pool.tile([128, GW, NH, HD + 1], bf)
                with nc.allow_low_precision(reason="bf16 ones col"):
                    nc.scalar.copy(
                        v65[:, :, :, HD : HD + 1],
                        ones_col[:].rearrange("p (g h) -> p g h", g=GW)[
                            :, :, :, None
                        ],
                    )

                for w in range(GW):
                    # V for this window: token-major, 8 heads per psum bank
                    for nk in range(2):
                        ps = psA.tile([128, 512], f32, tag="psA")
                        lo = 2 * C + 512 * nk
                        for k in range(KC):
                            nc.tensor.matmul(
                                ps[:],
                                xT[:, g, k, 128 * w : 128 * (w + 1)],
                                wq_sb[:, k, lo : lo + 512],
                                start=(k == 0),
                                stop=(k == KC - 1 and not has_qkvb),
                            )
                        if has_qkvb:
                            nc.tensor.matmul(
                                ps[:],
                                ones_tok[0:1, 0:M],
                                qkvb_sb[0:1, lo : lo + 512],
                                start=False,
                                stop=True,
                            )
                        with nc.allow_low_precision(reason="bf16 v65"):
                            nc.scalar.copy(
                                v65[:, w, 8 * nk : 8 * nk + 8, 0:HD],
                                ps[:].rearrange("p (h e) -> p h e", e=HD),
                            )

                    # S^T per head (bf16, ap=128), exp per 4-head bank
                    E = e_pool.tile([128, NH * M], bf)
                    for hb in range(4):
                        psS = psB.tile([128, 512], f32, tag="psB")
                        for m in range(4):
                            h = 4 * hb + m
                            nc.tensor.matmul(
                                psS[:, 128 * m : 128 * (m + 1)],
                                qkT[:, NH + h, 128 * w : 128 * (w + 1)],
                                qkT[:, h, 128 * w : 128 * (w + 1)],
                                start=True,
                                stop=True,
                            )
                        with nc.allow_low_precision(reason="bf16 attn weights"):
                            nc.scalar.activation(
                                E[:, 512 * hb : 512 * (hb + 1)], psS[:], Exp
                            )

                    # previous window's normalize+proj goes here so the PE
                    # has V/S work covering its ACT recip chain
                    if prev is not None:
                        finish(*prev)

                    # A.V: rows 0..63 unnormalized out, row 64 = denominator
                    psV = psV_pool.tile([128, 4 * 512], f32)
                    for h in range(NH):
                        nc.tensor.matmul(
                            psV[0:65, 128 * h : 128 * (h + 1)],
                            v65[:, w, h, :],
                            E[:, 128 * h : 128 * (h + 1)],
                            start=True,
                            stop=True,
                        )
                    # one Ln + one exp(-x) for all 16 heads' denominators
                    L = r_pool.tile([1, 4 * 512], f32, tag="L", bufs=1)
                    nc.scalar.activation(L[:], psV[64:65, :], Ln)
                    r4 = r_pool.tile([1, 4 * 512], fpr, tag="r4", bufs=2)
                    with nc.allow_low_precision(reason="f32r recip"):
                        nc.scalar.activation(r4[:], L[:], Exp, scale=-1.0)
                    prev = (g * GW + w, psV, r4)

            finish(*prev)

    _split_drain_waits(nc, mybir)
    return nc


def _get_nc(has_qkvb, has_projb):
    key = (has_qkvb, has_projb)
    if key not in _BUILD_CACHE:
        _BUILD_CACHE[key] = _build(has_qkvb, has_projb)
    return _BUILD_CACHE[key]


def _pack_weights(qkv_w, proj_w):
    # wqkv: [oc, p, k, j] from qkv_w.T[c, o]; c = k*128+p, o = oc*128+j
    wq = np.ascontiguousarray(
        qkv_w.T.astype(bf16).reshape(KC, 128, NOC, 128).transpose(2, 1, 0, 3)
    )
    wp = np.ascontiguousarray(
        proj_w.T.astype(bf16).reshape(KC, 128, C).transpose(1, 0, 2)
    )
    return wq, wp


def _pack_x_slab(xslab):
    # xslab [2048, C] tokens in (tt, ih, hh, iw, ww) order ->
    # [NGRP, 128 c-part, KC, TOKG] with windows (ih, iw) grouped by 4,
    # intra-window token (tt, hh, ww)
    xw = (
        xslab.reshape(WT, 4, WH, 4, WW, C)
        .transpose(1, 3, 0, 2, 4, 5)
        .reshape(NWIN, M, C)
        .astype(bf16)
    )
    # [win, tok, c] -> [g, p, k, w_in_g*128+tok]; c = k*128+p
    xt = (
        xw.reshape(NGRP, GW, M, KC, 128)
        .transpose(0, 4, 3, 1, 2)
        .reshape(NGRP, 128, KC, TOKG)
    )
    return np.ascontiguousarray(xt)


def _unpack_out(owin):
    # [NWIN(ih,iw), M(tt,hh,ww), C] -> [2048(tt,ih,hh,iw,ww), C]
    return (
        owin.reshape(4, 4, WT, WH, WW, C)
        .transpose(2, 0, 3, 1, 4, 5)
        .reshape(SLAB, C)
    )


def prepare_in_maps(x, qkv_w, qkv_b, proj_w, proj_b):
    has_qkvb = bool(np.any(qkv_b))
    has_projb = bool(np.any(proj_b))
    wq, wp = _pack_weights(qkv_w, proj_w)
    in_maps = []
    for core in range(NCORES):
        b, it = divmod(core, T // WT)
        im = {
            "xs": _pack_x_slab(x[b, it * SLAB : (it + 1) * SLAB, :]),
            "wqkv": wq,
            "wproj": wp,
        }
        if has_qkvb:
            im["qkvb"] = qkv_b.reshape(1, 3 * C).astype(bf16)
        if has_projb:
            im["projb"] = proj_b.reshape(1, C).astype(bf16)
        in_maps.append(im)
    return in_maps, has_qkvb, has_projb


def kernel(x, qkv_w, qkv_b, proj_w, proj_b, t, h, w, **_unused):
    from concourse.bass_utils import run_bass_kernel_spmd

    x = np.asarray(x, dtype=np.float32)
    qkv_w = np.asarray(qkv_w, dtype=np.float32)
    qkv_b = np.asarray(qkv_b, dtype=np.float32)
    proj_w = np.asarray(proj_w, dtype=np.float32)
    proj_b = np.asarray(proj_b, dtype=np.float32)
    assert x.shape == (B, N, C), x.shape
    assert int(t) == T and int(h) == H and int(w) == W

    in_maps, has_qkvb, has_projb = prepare_in_maps(
        x, qkv_w, qkv_b, proj_w, proj_b
    )
    nc = _get_nc(has_qkvb, has_projb)
    res = run_bass_kernel_spmd(nc, in_maps, core_ids=list(range(NCORES)))

    y = np.empty((B, N, C), dtype=np.float32)
    for core in range(NCORES):
        b, it = divmod(core, T // WT)
        y[b, it * SLAB : (it + 1) * SLAB, :] = _unpack_out(
            res.results[core]["out"]
        )
    return y
